# revision 25
# baseline (speedup 1.0000x reference)
"""Multi-head attention (B=2, H=8, S=2048, hd=16) on 8 Trainium2 NeuronCores.

Sharding: 16 (batch, head) attention groups -> 2 heads per core (cores 0-3:
batch 0, cores 4-7: batch 1).  Each core receives the (transposed) embeddings
for its batch, the 32 projection-weight columns for its two heads, and a
key-compacted copy of the embeddings (keys whose source mask is 0 contribute
exactly-zero softmax probability in fp32, so they are dropped; the compacted
set is padded with -1000 additive-mask columns which also exp to exactly 0).

All matmuls stream float32r (1 PE cycle/column vs 4 for exact fp32); the
bf16-level operand rounding keeps end-to-end error ~1e-2, inside the 2e-2
gate.  Per head the kernel runs a two-pass softmax:

  pass A ([q,k] layout, keys padded to NKA=1280 so every PSUM chunk is
    >=256 columns): S = (Q/4)K^T + mask via a 17-row contraction; a single
    fused DVE tensor_tensor_reduce per q-block computes
    min(-max(sc_lo, sc_hi)) = -rowmax in one 640-element pass.
  pass B ([k,q] layout, NK=1152): S^T - rowmax via an 18-row contraction
    (16 dims + mask*ones + ones*(-rowmax)); ACT exp -> P^T; ctx accumulates
    P^T @ [V | 1] in PSUM, where the ones column yields the softmax
    denominator l.  Final scale by 1/l uses a DMA partition-broadcast and a
    Pool-engine multiply.

The B pipeline is software-pipelined (st/exp of iteration i+1 issued before
ctx of iteration i) and pass A of later (head, q-half) groups is injected
one q-block per B iteration so the PE never idles waiting for DVE reduces.
PSUM: one shared 2-buf pool (3 banks per buf) rotates st [128,1024]f32 and
sc [128,1280]f32 tiles; ctx [17,1024] holds the remaining 2 banks.

Output per core is a dense [32, 2048] (dim-major) tensor; the host scatters
columns back into the interleaved head layout (out[..., d*8+h] = ctx[d]).
"""

import numpy as np

S = 2048
E = 128
HD = 16
NK = 1152            # pass-B padded compacted key count (9 blocks of 128)
NKA = 1280           # pass-A padded key count (chunks 512/512/256, all >=256)
NKB = NK // 128
NQB = S // 128
NEG = -1000.0

_PROG = None


def _build_program():
    import concourse.mybir as mybir
    from concourse import bacc
    from concourse.tile import TileContext

    fp32 = mybir.dt.float32
    f32r = mybir.dt.float32r
    AF = mybir.ActivationFunctionType
    ALU = mybir.AluOpType
    AX = mybir.AxisListType

    nc = bacc.Bacc()

    xT = nc.declare_dram_parameter("xT", [E, S], f32r, isOutput=False)
    xkT = nc.declare_dram_parameter("xkT", [E, NKA], f32r, isOutput=False)
    # weight columns padded to 48: head0 dims at 0:16, head1 dims at 32:48
    # (PSUM partition slices must start at 0/32/64/96); wq pre-scaled by 0.25
    wq = nc.declare_dram_parameter("wq", [E, 48], f32r, isOutput=False)
    wk = nc.declare_dram_parameter("wk", [E, 48], f32r, isOutput=False)
    wv = nc.declare_dram_parameter("wv", [E, 48], f32r, isOutput=False)
    maskrow = nc.declare_dram_parameter("maskrow", [1, NKA], f32r, isOutput=False)
    onesr = nc.declare_dram_parameter("onesr", [1, S], f32r, isOutput=False)
    ident = nc.declare_dram_parameter("ident", [E, E], fp32, isOutput=False)
    out_d = nc.declare_dram_parameter("out", [2 * HD, S], fp32, isOutput=True)
    ldram = nc.dram_tensor("ldram", [2, S], fp32)

    with TileContext(nc) as tc:
        with (
            tc.tile_pool(name="consts", bufs=1) as cpool,
            tc.tile_pool(name="work", bufs=1) as wpool,
            tc.tile_pool(name="ptp", bufs=3) as ptpool,
            tc.tile_pool(name="stp", bufs=3, space="PSUM") as stpool,
            tc.tile_pool(name="scp", bufs=1, space="PSUM") as scpool,
            tc.tile_pool(name="ctxp", bufs=2, space="PSUM") as ctxpool,
        ):
            # ---------------- constant loads (weights first: shortest
            # critical path to the first projection matmul) ----------------
            wk_sb = cpool.tile([E, 48], f32r, name="wk_sb")
            nc.sync.dma_start(out=wk_sb[:, :], in_=wk[:, :])
            wq_sb = cpool.tile([E, 48], f32r, name="wq_sb")
            nc.sync.dma_start(out=wq_sb[:, :], in_=wq[:, :])
            wv_sb = cpool.tile([E, 48], f32r, name="wv_sb")
            nc.sync.dma_start(out=wv_sb[:, :], in_=wv[:, :])
            xkT_sb = cpool.tile([E, NKA], f32r, name="xkT_sb")
            for o, n in ((0, 512), (512, 512), (1024, 256)):
                nc.sync.dma_start(out=xkT_sb[:, o : o + n], in_=xkT[:, o : o + n])
            xT_sb = cpool.tile([E, S], f32r, name="xT_sb")
            for o in range(0, S, 512):
                nc.sync.dma_start(out=xT_sb[:, o : o + 512], in_=xT[:, o : o + 512])
            ident_sb = cpool.tile([E, E], fp32, name="ident_sb")
            nc.sync.dma_start(out=ident_sb[:, :], in_=ident[:, :])

            # ---------------- persistent work tensors ----------------
            # Both heads stacked at partition offsets 0 / 32 so one PSUM-evac
            # copy serves both heads (engine cost is free-dim only).
            # qtall rows per head h (base b=32h): b..b+15 q-dims (x0.25),
            #   b+16 ones, b+17 -rowmax.  ktall rows: k-dims, mask, ones.
            qtall = wpool.tile([50, S], f32r, name="qtall")
            ktall = wpool.tile([50, NKA], f32r, name="ktall")
            vv = [wpool.tile([128, NKB, HD + 1], f32r, name=f"vv{h}") for h in range(2)]
            negc = [wpool.tile([128, NQB], fp32, name=f"negc{h}") for h in range(2)]
            nT_sb = [wpool.tile([4, 4, 128], f32r, name=f"nT_sb{h}") for h in range(2)]
            ctxl = wpool.tile([49, S], fp32, name="ctxl")
            ones16 = wpool.tile([1, 16], f32r, name="ones16")
            nc.sync.dma_start(out=ones16[:, :], in_=onesr[0:1, 0:16])
            ldual = wpool.tile([33, S], fp32, name="ldual")
            linv0 = wpool.tile([1, S], fp32, name="linv0")
            linv1 = wpool.tile([1, S], f32r, name="linv1")
            lbc = wpool.tile([48, S], fp32, name="lbc")
            out_sb = wpool.tile([64, S], fp32, name="out_sb")

            # ---------------- projections: KT, QT, V ----------------
            for o, n in ((0, 512), (512, 512), (1024, 256)):
                kt_ps = stpool.tile([48, 512], fp32, name="kt_ps", tag="st")
                nc.tensor.matmul(
                    kt_ps[:, 0:n],
                    lhsT=wk_sb[:, :],
                    rhs=xkT_sb[:, o : o + n],
                    start=True,
                    stop=True,
                )
                nc.scalar.copy(ktall[0:48, o : o + n], kt_ps[:, 0:n])
            for h in range(2):
                nc.sync.dma_start(
                    out=ktall[32 * h + 16 : 32 * h + 17, :], in_=maskrow[:, :]
                )
                nc.sync.dma_start(
                    out=ktall[32 * h + 17 : 32 * h + 18, 0:NK], in_=onesr[:, 0:NK]
                )

            for half in range(4):
                qt_ps = stpool.tile([48, 512], fp32, name="qt_ps", tag="st")
                nc.tensor.matmul(
                    qt_ps[:, :],
                    lhsT=wq_sb[:, :],
                    rhs=xT_sb[:, 512 * half : 512 * (half + 1)],
                    start=True,
                    stop=True,
                )
                nc.scalar.copy(
                    qtall[0:48, 512 * half : 512 * (half + 1)], qt_ps[:, :]
                )
            for h in range(2):
                nc.sync.dma_start(
                    out=qtall[32 * h + 16 : 32 * h + 17, :], in_=onesr[:, :]
                )

            # V projection: all key blocks into one PSUM tile, then one
            # strided copy per head; ones column via DMA broadcast.
            v_ps = stpool.tile([128, NKB, 48], fp32, name="v_ps", tag="st")
            for kb in range(NKB):
                nc.tensor.matmul(
                    v_ps[:, kb, :],
                    lhsT=xkT_sb[:, 128 * kb : 128 * (kb + 1)],
                    rhs=wv_sb[:, :],
                    start=True,
                    stop=True,
                )
            for h in range(2):
                nc.scalar.copy(vv[h][:, :, 0:HD], v_ps[:, :, 32 * h : 32 * h + 16])
                nc.sync.dma_start(
                    out=vv[h][:, :, HD : HD + 1],
                    in_=onesr[0:1, 0:NKB].to_broadcast([128, NKB]),
                )

            # ---------------- pass A: -rowmax per q-block ----------------
            def a_iter(h, qb):
                sc = scpool.tile([128, NKA], fp32, name="sc", tag="sc")
                lhs = qtall[32 * h : 32 * h + 17, 128 * qb : 128 * (qb + 1)]
                for o, n in ((0, 512), (512, 512), (1024, 256)):
                    nc.tensor.matmul(
                        sc[:, o : o + n],
                        lhsT=lhs,
                        rhs=ktall[32 * h : 32 * h + 17, o : o + n],
                        start=True,
                        stop=True,
                    )
                # pad columns NK: are all -1000 and rowmax >= 0 here, so the
                # reduce only needs the first NK columns
                nc.vector.tensor_reduce(
                    negc[h][:, qb : qb + 1],
                    sc[:, 0:NK],
                    axis=AX.X,
                    op=ALU.max,
                    negate=True,
                )

            # negm at 512-column (4 q-block) granularity: the B stream for
            # column-group cg only needs its own quarter of row 17.
            def negm_assemble(h, cg):
                ntp = stpool.tile([4, 128], fp32, name="ntp", tag="st")
                nc.tensor.transpose(
                    ntp[:, :], negc[h][:, 4 * cg : 4 * (cg + 1)], ident_sb[:, :]
                )
                eng = nc.scalar.copy if h == 0 else (
                    lambda out, in_: nc.vector.tensor_copy(out=out, in_=in_)
                )
                eng(out=nT_sb[h][:, cg, :], in_=ntp[:, :])
                nc.sync.dma_start(
                    out=qtall[
                        32 * h + 17 : 32 * h + 18, 512 * cg : 512 * (cg + 1)
                    ].rearrange("a (b f) -> a b f", b=4),
                    in_=nT_sb[h][:, cg, :],
                )

            # ---------------- pass B pipeline pieces ----------------
            def st_exp(h, cg, kb):
                st = stpool.tile([128, 512], fp32, name="st", tag="st")
                nc.tensor.matmul(
                    st[:, :],
                    lhsT=ktall[32 * h : 32 * h + 18, 128 * kb : 128 * (kb + 1)],
                    rhs=qtall[32 * h : 32 * h + 18, 512 * cg : 512 * (cg + 1)],
                    start=True,
                    stop=True,
                )
                pt = ptpool.tile([128, 512], f32r, name="pt", tag="pt")
                nc.scalar.activation(pt[:, :], st[:, :], AF.Exp)
                return pt

            def ctx_acc(h, kb, ctxc, pt):
                nc.tensor.matmul(
                    ctxc[0:17, :],
                    lhsT=vv[h][:, kb, :],
                    rhs=pt[:, :],
                    start=(kb == 0),
                    stop=(kb == NKB - 1),
                )

            def evac(h, cg, ctxc):
                nc.scalar.copy(
                    ctxl[32 * h : 32 * h + 17, 512 * cg : 512 * (cg + 1)],
                    ctxc[0:17, :],
                )

            def finals_h0():
                # fully hidden mid-stream: DMA broadcast + Pool multiply
                nc.sync.dma_start(out=ldual[0:1, :], in_=ctxl[16:17, :])
                nc.vector.reciprocal(linv0[0:1, :], ldual[0:1, :])
                nc.sync.dma_start(out=ldram[0:1, :], in_=linv0[0:1, :])
                nc.sync.dma_start(
                    out=lbc[0:16, :], in_=ldram[0:1, :].to_broadcast([HD, S])
                )
                nc.gpsimd.tensor_tensor(
                    out=out_sb[0:16, :],
                    in0=ctxl[0:16, :],
                    in1=lbc[0:16, :],
                    op=ALU.mult,
                )
                nc.sync.dma_start(out=out_d[0:16, :], in_=out_sb[0:16, :])

            def finals_h1():
                # latency-lean tail: PE broadcast into PSUM + DVE multiply
                nc.sync.dma_start(out=ldual[32:33, :], in_=ctxl[48:49, :])
                with nc.allow_low_precision(reason="1/l row kept in f32r for PE broadcast"):
                    nc.vector.reciprocal(linv1[0:1, :], ldual[32:33, :])
                for c in range(4):
                    lb = stpool.tile([16, 512], fp32, name="lb", tag="st")
                    nc.tensor.matmul(
                        lb[:, :],
                        lhsT=ones16[:, :],
                        rhs=linv1[0:1, 512 * c : 512 * (c + 1)],
                        start=True,
                        stop=True,
                    )
                    nc.vector.tensor_tensor(
                        out=out_sb[32:48, 512 * c : 512 * (c + 1)],
                        in0=ctxl[32:48, 512 * c : 512 * (c + 1)],
                        in1=lb[:, :],
                        op=ALU.mult,
                    )
                nc.sync.dma_start(out=out_d[16:32, :], in_=out_sb[32:48, :])

            # ---------------- schedule ----------------
            # Prologue: pass A for (h0, cg0) only -- 4 q-blocks.
            for qb in range(4):
                a_iter(0, qb)
            negm_assemble(0, 0)

            inject = [(0, qb) for qb in range(4, 16)] + [
                (1, qb) for qb in range(16)
            ]
            # negm(h, cg) fires right after the injection that completes its
            # 4 q-blocks
            negm_after = {}
            for k in range(7):
                grp = k + 1  # groups (0,1)..(0,3),(1,0)..(1,3)
                negm_after[4 * k + 3] = (grp // 4, grp % 4)
            # iteration indices at which to inject: first 4 every iter (cg1
            # urgency), then every other iter
            inj_iters = list(range(4)) + list(range(4, 4 + 2 * len(inject), 2))

            phases = [(h, cg) for h in range(2) for cg in range(4)]
            prev = None
            ii = 0
            it = 0
            for h, cg in phases:
                ctxc = ctxpool.tile([17, 512], fp32, name="ctx", tag="ctx")
                for kb in range(NKB):
                    pt = st_exp(h, cg, kb)
                    if ii < len(inject) and it >= inj_iters[ii]:
                        fired = ii
                        a_iter(*inject[ii])
                        ii += 1
                        if fired in negm_after:
                            negm_assemble(*negm_after[fired])
                    if prev is not None:
                        ph, pcg, pkb, pctxc, ppt = prev
                        ctx_acc(ph, pkb, pctxc, ppt)
                        if pkb == NKB - 1:
                            evac(ph, pcg, pctxc)
                            if pcg == 3 and ph == 0:
                                finals_h0()
                    prev = (h, cg, kb, ctxc, pt)
                    it += 1
            ph, pcg, pkb, pctxc, ppt = prev
            ctx_acc(ph, pkb, pctxc, ppt)
            evac(ph, pcg, pctxc)
            finals_h1()

    nc.finalize()
    return nc


def _prep_core_inputs(x, msk_add_full, w_query, w_key, w_value):
    """Build the 8 per-core input maps from full inputs."""
    B = x.shape[0]
    in_maps = []
    onesrow = np.ones((1, S), dtype=np.float32)
    identm = np.eye(E, dtype=np.float32)
    per_batch = []
    for b in range(B):
        keep = np.flatnonzero(msk_add_full[b] == 0.0)
        nk = len(keep)
        assert 0 < nk <= NK, f"compacted key count {nk} out of range"
        xk = np.zeros((NKA, E), dtype=np.float32)
        xk[:nk] = x[b][keep]
        maskrow = np.full((1, NKA), NEG, dtype=np.float32)
        maskrow[0, :nk] = 0.0
        xTb = np.ascontiguousarray(x[b].T)
        xkTb = np.ascontiguousarray(xk.T)
        per_batch.append((xTb, xkTb, maskrow))
    for c in range(8):
        b = c // 4
        h0 = 2 * (c % 4)
        xTb, xkTb, maskrow = per_batch[b]

        def _pad48(w, scale=1.0):
            wc = np.zeros((E, 48), dtype=np.float32)
            wc[:, 0:16] = w[:, h0::8] * scale
            wc[:, 32:48] = w[:, h0 + 1 :: 8] * scale
            return wc

        in_maps.append(
            {
                "xT": xTb,
                "xkT": xkTb,
                "wq": _pad48(w_query, 0.25),
                "wk": _pad48(w_key),
                "wv": _pad48(w_value),
                "maskrow": maskrow,
                "onesr": onesrow,
                "ident": identm,
            }
        )
    return in_maps


def kernel(
    input_embeddings,
    token_attention_masks_source,
    token_attention_masks_target,
    masked,
    w_query,
    w_key,
    w_value,
):
    global _PROG
    x = np.asarray(input_embeddings, dtype=np.float32)
    msk = np.asarray(token_attention_masks_source)
    wq_f = np.asarray(w_query, dtype=np.float32)
    wk_f = np.asarray(w_key, dtype=np.float32)
    wv_f = np.asarray(w_value, dtype=np.float32)
    assert int(np.asarray(masked)) == 0, "only the encoder (masked=0) path is supported"
    B = x.shape[0]
    assert x.shape == (2, S, E)

    msk_add = np.where(msk == 0, np.float32(NEG), np.float32(0.0))
    in_maps = _prep_core_inputs(x, msk_add, wq_f, wk_f, wv_f)

    if _PROG is None:
        _PROG = _build_program()
    nc = _PROG

    from concourse.bass_utils import run_bass_kernel_spmd

    res = run_bass_kernel_spmd(nc, in_maps, list(range(8)))

    out = np.empty((B, S, E), dtype=np.float32)
    for c in range(8):
        b = c // 4
        h0 = 2 * (c % 4)
        o = res.results[c]["out"]  # [32, 2048]
        out[b][:, h0::8] = o[0:16, :].T
        out[b][:, h0 + 1 :: 8] = o[16:32, :].T
    return out
